# revision 14
# baseline (speedup 1.0000x reference)
"""Fused causal multi-head attention block (QKV proj + causal attention +
out proj) for TRN2, data-parallel over batch across 8 NeuronCores.

Per-core layout strategy (batch element b on core b):
  - qkT [1536,1024] = (q|k) projection computed directly transposed
    (head_dim on partitions). q rows (weights + bias) pre-scaled by 1/8 on
    host. q head pair 2p/2p+1 shares one 128-partition tile. Only the k
    side is zero-padded per head (stationary operand of the S^T matmul):
    the full-K=128 product contracts to a single head through the zeros,
    keeping the HAM clock gate at 8/8.
  - Attention computes S^T = K Q^T blocks directly (keys on partitions):
    softmax numerators exp(S^T) land in the P^T layout the AV matmul
    needs, with zero on-chip transposes. No max shift (scores are O(9)).
    Causal masking is a GpSimd affine_select on the diagonal block after
    the exp.
  - 64 ones columns interleaved in the AV stationary emit the softmax
    denominators on PSUM partitions 64..127; an Act ln->exp reciprocal
    (one table set with Exp) + one DVE multiply normalize attn_out^T
    during its PSUM->SBUF copyback.
  - attn_out^T is accumulated per head in [d, t] layout = proj lhsT
    directly. v bias folded into an effective proj bias on host.

Engine-queue findings this schedule is built around (measured):
  - dma_start instructions BLOCK their issuing engine queue until the
    data lands (~0.6us fixed + bytes/BW each), and only SP (sync), Act
    (scalar) and GpSimd can issue DMAs. So: big single-DMA loads, spread
    over the three queues by when each consumer needs data, and the Act
    queue carries only the v weights (done ~13us) so the exp stream is
    never blocked behind input streaming (the old layout had x on Act,
    pinning the first exp past the last x byte at ~18us).
  - The PE clock ramps from half speed over the first ~9us of activity
    and no DMA data can arrive before ~9us (fixed engine-start +
    descriptor latency): a stream of dummy warmup matmuls on a
    GpSimd-memset tile burns the ramp inside that dead window.
  - qkb is pre-transposed on host to [128,12] so its DMA is a tiny
    contiguous copy (the strided gather version landed ~14us late and
    gated the whole attention stream through the q2[0] bias add).
  - k weights split so the [128-col] block feeding heads 0/1 lands first:
    S(0) AND S(1) need only qk(0)+qk(6), so the Act exp stream starts
    ~21us with two heads of runway before qk(1)/qk(7) even finish.

All matmul operands fp16 (1 cycle/row on PE vs 4 for fp32), fp32
accumulation in PSUM, softmax stats in fp32.
"""

import contextlib

import numpy as np

import concourse.bass as bass
import concourse.mybir as mybir
import concourse.tile as tile
from concourse.bass_utils import run_bass_kernel_spmd

B, N, C, H = 8, 1024, 768, 12
HD = C // H
HP = H // 2           # 6 head pairs
HDS = 2 * HD          # AV stationary width: head_dim + ones columns
SCALE = HD ** -0.5
P = 128
NT = N // P           # 8 token tiles
KC = C // P           # 6 contraction tiles over C
MOQ = 2 * C // P      # 12 output tiles of the qk projection
F32 = mybir.dt.float32
F16 = mybir.dt.float16
NPF16 = np.float16

MM_CHUNK = 512        # max matmul moving size this walrus accepts


def _patch_tile_drain():
    """This walrus caps sync waits at 1 per non-EventSemaphore instruction;
    TileContext._drain_and_barrier packs all outstanding waits onto the tail
    drain. Spread them over standalone wait instructions instead."""
    if getattr(tile.TileContext, "_drain_patched", False):
        return
    from concourse.vector_clock import ScopedClock

    def _drain_and_barrier(self, tick_clock, wait_clock):
        nc = self.nc
        probe = mybir.InstNoOp(name=nc.get_next_instruction_name(), ins=[], outs=[])
        probe.engine = mybir.EngineType.SP
        wait_clock.add_sem_waits(probe, ScopedClock({None: tick_clock.global_clock}))
        si = probe.sync_info
        by_name = {h.name: h for h in self.sems.allocated().values()}
        by_num = {h.num: h for h in self.sems.allocated().values()}
        for w in list(si.on_wait or []) if si is not None else []:
            sem = by_name.get(w.ant_name) or by_num.get(w.id)
            assert sem is not None, f"unknown sem {w.ant_name} id={w.id}"
            nc.sync.wait_ge(sem, w.wait_value)
        nc.sync.drain()
        nc.all_engine_barrier()
        assert self.sems is not None
        popped = nc._tile_sem_poison_stack.pop()
        assert popped is self._sem_poison
        nc.clear_and_free_semaphores(list(self.sems.allocated().values()))
        nc.all_engine_barrier()

    tile.TileContext._drain_and_barrier = _drain_and_barrier
    tile.TileContext._drain_patched = True


def _split_excess_waits(nc, max_waits=1):
    """Move excess per-instruction sem waits onto preceding same-engine NoOps
    (this walrus rejects >1 wait on most instruction encodings)."""
    for f in nc.m.functions:
        for bb in f.blocks:
            new = []
            changed = False
            for inst in bb.instructions:
                si = inst.sync_info
                waits = list(si.on_wait) if si is not None and si.on_wait else []
                cap = 2 if isinstance(inst, mybir.InstEventSemaphore) else max_waits
                if len(waits) > cap:
                    changed = True
                    for w in waits[:-cap]:
                        nop = mybir.InstNoOp(
                            name=f"I-wsplit-{nc.next_id()}", ins=[], outs=[]
                        )
                        nop.engine = inst.engine
                        nop.sync_info = mybir.SyncInfo(on_wait=[w], on_update=[])
                        new.append(nop)
                    inst.sync_info = mybir.SyncInfo(
                        on_wait=waits[-cap:], on_update=list(si.on_update or [])
                    )
                new.append(inst)
            if changed:
                bb.instructions = new


def _chunks(total, start=0, chunk=MM_CHUNK):
    out = []
    pos = start
    while pos < total:
        w = min(chunk, total - pos)
        out.append((pos, w))
        pos += w
    return out


def build():
    nc = bass.Bass("TRN2", target_bir_lowering=False, debug=False)

    xT = nc.dram_tensor("xT", [C, N], F16, kind="ExternalInput").ap()
    qkwT = nc.dram_tensor("qkwT", [C, 2 * C], F16, kind="ExternalInput").ap()
    vwT = nc.dram_tensor("vwT", [C, C], F16, kind="ExternalInput").ap()
    pwT = nc.dram_tensor("pwT", [C, C], F16, kind="ExternalInput").ap()
    qkb2 = nc.dram_tensor("qkb2", [P, MOQ], F32, kind="ExternalInput").ap()
    pb = nc.dram_tensor("pb", [C], F32, kind="ExternalInput").ap()
    y = nc.dram_tensor("y", [N, C], F32, kind="ExternalOutput").ap()

    with tile.TileContext(nc) as tc, contextlib.ExitStack() as ctx:
        const = ctx.enter_context(tc.tile_pool(name="const", bufs=1))
        wpool = ctx.enter_context(tc.tile_pool(name="w", bufs=1))
        apool = ctx.enter_context(tc.tile_pool(name="acts", bufs=1))
        stat = ctx.enter_context(tc.tile_pool(name="stat", bufs=4))
        ypool = ctx.enter_context(tc.tile_pool(name="y", bufs=2))
        psS = ctx.enter_context(tc.tile_pool(name="psS", bufs=3, space="PSUM"))
        psAV = ctx.enter_context(tc.tile_pool(name="psAV", bufs=2, space="PSUM"))

        pb_t = const.tile([P, C], F32)
        qkb_t = const.tile([P, MOQ], F32)
        wu_t = const.tile([P, 640], F16)

        qh_t = wpool.tile([P, KC, C], F16)        # q weights (mo 0..5)
        khA_t = wpool.tile([P, KC, P], F16)       # k weights mo 6 (heads 0/1)
        khB_t = wpool.tile([P, KC, C - P], F16)   # k weights mo 7..11
        xh_t = [
            wpool.tile([P, KC, MM_CHUNK], F16, name=f"xh{i}", tag=f"xh{i}")
            for i in range(2)
        ]
        vw_t = wpool.tile([P, KC, C], F16)
        pw_t = wpool.tile([P, KC, C], F16)

        qkw_r = qkwT.rearrange("(k p) o -> p k o", p=P)
        x_r = xT.rearrange("(k p) o -> p k o", p=P)
        # sync queue: per-DMA the queue blocks until data lands, so order =
        # consumer order. khA (tiny) + qh feed qk(0)/qk(6) -> S(0)/S(1).
        nc.sync.dma_start(out=khA_t, in_=qkw_r[:, :, C : C + P])
        nc.sync.dma_start(out=qh_t, in_=qkw_r[:, :, 0:C])
        nc.sync.dma_start(out=qkb_t, in_=qkb2)
        nc.sync.dma_start(out=khB_t, in_=qkw_r[:, :, C + P : 2 * C])
        nc.sync.dma_start(out=pw_t, in_=pwT.rearrange("(k p) o -> p k o", p=P))
        nc.sync.dma_start(
            out=pb_t,
            in_=bass.AP(tensor=pb.tensor, offset=pb.offset, ap=[[0, P]] + list(pb.ap)),
        )
        # Act queue: only vw (needed ~20us), then the act table load + exps
        nc.scalar.dma_start(out=vw_t, in_=vwT.rearrange("(k p) o -> p k o", p=P))

        def qkw_st(mo, kc):
            # stationary [128,128] block of the qk weight for output tile mo
            if mo < KC:
                return qh_t[:, kc, mo * P : (mo + 1) * P]
            if mo == KC:
                return khA_t[:, kc, :]
            c0 = (mo - KC - 1) * P
            return khB_t[:, kc, c0 : c0 + P]

        def x_mv(kc, t0, tw):
            h = t0 // MM_CHUNK
            o = t0 - h * MM_CHUNK
            return xh_t[h][:, kc, o : o + tw]

        # q tiles shared per head pair; k tiles zero-padded per head. Both
        # rotate through small pools (lifetimes span <=5 rounds) to pay for
        # the 4th exp buffer below.
        q2pool = ctx.enter_context(tc.tile_pool(name="q2p", bufs=4))
        kppool = ctx.enter_context(tc.tile_pool(name="kpp", bufs=6))
        q2 = {}        # head pair -> tile
        kpad = {}      # head -> tile
        v_mt = [
            apool.tile([P, H, HDS], F16, name=f"v{mt}", tag=f"v{mt}")
            for mt in range(NT)
        ]
        attnT_t = apool.tile([P, KC, N], F16)      # attention output^T
        ptbufs = [
            apool.tile([P, NT, N], F16, tag=f"ptb{i}", name=f"ptb{i}")
            for i in range(4)
        ]  # exp(S^T) per head; 4 bufs so S(h+3) can run 3 heads ahead of
        # the AV consumer without WAR-clobbering a pending buffer
        PTB = len(ptbufs)

        # per-partition head-half selectors {1,0}/{0,1} for the k copyback
        mask01 = const.tile([P, 2], F32)
        nc.gpsimd.memset(mask01, 1.0)
        nc.gpsimd.affine_select(
            out=mask01[:, 0:1], in_=mask01[:, 0:1],
            compare_op=mybir.AluOpType.is_ge, fill=0.0,
            base=HD - 1, pattern=[[1, 1]], channel_multiplier=-1,
        )
        nc.gpsimd.affine_select(
            out=mask01[:, 1:2], in_=mask01[:, 1:2],
            compare_op=mybir.AluOpType.is_ge, fill=0.0,
            base=-HD, pattern=[[1, 1]], channel_multiplier=1,
        )
        # warmup operand, then x halves ride GpSimd's DGE, then the per-tile
        # ones columns for the AV denominators (all ahead of the causal
        # selects in this queue, all done before their consumers need them)
        nc.gpsimd.memset(wu_t, 0.0078125)
        nc.gpsimd.dma_start(out=xh_t[0], in_=x_r[:, :, 0:MM_CHUNK])
        nc.gpsimd.dma_start(out=xh_t[1], in_=x_r[:, :, MM_CHUNK:N])
        for mt in range(NT):
            nc.gpsimd.memset(v_mt[mt][:, :, HD:HDS], 1.0)

        # ---- PE warmup: dummy matmuls ramp the PE clock out of its
        # half-speed p-state during the ~13us window before operands land
        wps = psS.tile([P, MM_CHUNK], F32, tag="mm", name="wps")
        for i in range(12):
            nc.tensor.matmul(wps, wu_t[:, 0:P], wu_t[:, P:640], start=True, stop=(i == 11))
        for i in range(4):
            nc.tensor.matmul(
                wps[:, 0:P], wu_t[:, 0:P], wu_t[:, P : 2 * P], start=True, stop=True
            )

        # ---- emitters -------------------------------------------------
        def emit_qk(mo):
            ps = psS.tile([P, N], F32, tag="mm", name="ps_qk")
            for kc in range(KC):
                for t0, tw in _chunks(N):
                    nc.tensor.matmul(
                        ps[:, t0 : t0 + tw],
                        qkw_st(mo, kc),
                        x_mv(kc, t0, tw),
                        start=(kc == 0),
                        stop=(kc == KC - 1),
                    )
            if mo < KC:
                qt = q2pool.tile([P, N], F16, tag="q2", name="q2t")
                q2[mo] = qt
                nc.vector.tensor_scalar_add(qt, ps, qkb_t[:, mo : mo + 1])
            else:
                p = mo - KC
                for i in range(2):
                    kt = kppool.tile([P, N], F16, tag="kp", name="kpt")
                    kpad[2 * p + i] = kt
                    nc.vector.tensor_scalar(
                        out=kt,
                        in0=ps,
                        scalar1=qkb_t[:, mo : mo + 1],
                        scalar2=mask01[:, i : i + 1],
                        op0=mybir.AluOpType.add,
                        op1=mybir.AluOpType.mult,
                    )

        def emit_v(mt):
            ps = psS.tile([P, C], F32, tag="mm", name="ps_v")
            for kc in range(KC):
                for o0, ow in _chunks(C):
                    nc.tensor.matmul(
                        ps[:, o0 : o0 + ow],
                        x_mv(kc, mt * P, P),
                        vw_t[:, kc, o0 : o0 + ow],
                        start=(kc == 0),
                        stop=(kc == KC - 1),
                    )
            nc.vector.tensor_copy(
                out=v_mt[mt][:, :, 0:HD], in_=ps.rearrange("p (h d) -> p h d", h=H)
            )

        def emit_s(h, js):
            ptb = ptbufs[h % PTB]
            for j in js:
                t_lo = j * P
                s_ps = psS.tile([P, N], F32, tag="mm", name="s_ps")
                if t_lo % MM_CHUNK == 0:
                    regions = _chunks(N, start=t_lo)
                else:
                    nb = (t_lo // MM_CHUNK + 1) * MM_CHUNK
                    regions = [(t_lo, nb - t_lo)] + _chunks(N, start=nb)
                for t0, tw in regions:
                    nc.tensor.matmul(
                        s_ps[:, t0 : t0 + tw],
                        kpad[h][:, t_lo : t_lo + P],
                        q2[h // 2][:, t0 : t0 + tw],
                        start=True,
                        stop=True,
                    )
                nc.scalar.activation(
                    ptb[:, j, t_lo:],
                    s_ps[:, t_lo:],
                    mybir.ActivationFunctionType.Exp,
                )
                # causal mask: zero keys n > queries t in the diagonal block
                nc.gpsimd.affine_select(
                    out=ptb[:, j, t_lo : t_lo + P],
                    in_=ptb[:, j, t_lo : t_lo + P],
                    compare_op=mybir.AluOpType.is_ge,
                    fill=0.0,
                    base=0,
                    pattern=[[1, P]],
                    channel_multiplier=-1,
                )

        def emit_av_mm(h, ci):
            ptb = ptbufs[h % PTB]
            c0, cw = _chunks(N)[ci]
            av = psAV.tile([HDS, MM_CHUNK], F32, tag="av", name="av")
            js = [j for j in range(NT) if j * P < c0 + cw]
            for idx, j in enumerate(js):
                t0 = max(c0, j * P)
                nc.tensor.matmul(
                    av[:, t0 - c0 : cw],
                    v_mt[j][:, h, :],
                    ptb[:, j, t0 : c0 + cw],
                    start=(idx == 0),
                    stop=(idx == len(js) - 1),
                )
            return av

        def emit_av_norm(h, ci, av):
            po = (h % 2) * HD
            c0, cw = _chunks(N)[ci]
            # rows HD..2*HD hold the softmax denominators: Act ln->exp
            # reciprocal (one table set with Exp) + one DVE multiply
            ld = stat.tile([HD, MM_CHUNK], F32, tag="ld", name="ld")
            rb = stat.tile([HD, MM_CHUNK], F32, tag="rb", name="rb")
            nc.scalar.activation(
                ld[:, :cw],
                av[HD : 2 * HD, :cw],
                mybir.ActivationFunctionType.Ln,
            )
            nc.scalar.activation(
                rb[:, :cw],
                ld[:, :cw],
                mybir.ActivationFunctionType.Exp,
                scale=-1.0,
            )
            nc.vector.tensor_mul(
                attnT_t[po : po + HD, h // 2, c0 : c0 + cw],
                av[:HD, :cw],
                rb[:, :cw],
            )

        def emit_proj(mt):
            ps = psS.tile([P, C], F32, tag="mm", name="ps_y")
            for kc in range(KC):
                for o0, ow in _chunks(C):
                    nc.tensor.matmul(
                        ps[:, o0 : o0 + ow],
                        attnT_t[:, kc, mt * P : (mt + 1) * P],
                        pw_t[:, kc, o0 : o0 + ow],
                        start=(kc == 0),
                        stop=(kc == KC - 1),
                    )
            yt = ypool.tile([P, C], F32)
            nc.vector.tensor_add(yt, ps, pb_t)
            nc.sync.dma_start(out=y[mt * P : (mt + 1) * P, :], in_=yt)

        # ---- braided schedule. S(0)/S(1) need only qk(0)+qk(6), so the
        # Act exp stream gets two heads of runway immediately. AV chunk 0
        # (tokens 0..511) touches only key blocks j<=3, so it runs in round
        # h while chunk 1 (which needs every v tile) lags one round —
        # letting v4..v7 and the remaining qk projections spread into the
        # early rounds instead of forming a PE wall before round 0. With 4
        # exp buffers S(h+3) runs right after av1(h-1) releases its buffer,
        # keeping Act three heads ahead of the AV consumer. ----
        with nc.named_scope("head_start"):
            emit_qk(6)
            emit_qk(0)
            emit_v(0)
            emit_s(0, [0, 1, 2, 3])
            emit_v(1)
            emit_s(0, [4, 5, 6, 7])
            emit_s(1, [0, 1, 2, 3])
            emit_qk(7)
            emit_s(1, [4, 5, 6, 7])
            emit_qk(1)
            emit_v(2)
            emit_s(2, [0, 1, 2, 3, 4])
            emit_v(3)
            emit_s(2, [5, 6, 7])

        # round h: av0(h), av1(h-1), S(h+3), fillers. qk fillers are
        # consumed by S two-to-three rounds later; v tiles 4..7 must all
        # land before av1(0) in round 1.
        fillers = {0: [8, 2], 1: [9], 2: [3], 3: [10], 4: [4], 5: [11], 6: [5]}
        extra_v = {0: [4], 1: [5, 6, 7]}
        for h in range(H):
            with nc.named_scope(f"round{h}"):
                av0 = emit_av_mm(h, 0)
                emit_av_norm(h, 0, av0)
                if h == 0:
                    # S(3) -> ptb[3] has no prior user, safe before any av1
                    emit_s(3, [0, 1, 2])
                for mo in fillers.get(h, [])[:1]:
                    emit_qk(mo)
                for mt in extra_v.get(h, []):
                    emit_v(mt)
                if h >= 1:
                    av1 = emit_av_mm(h - 1, 1)
                    emit_av_norm(h - 1, 1, av1)
                # S(h+3) writes the buffer av1(h-1) just released (stride-4
                # reuse), so its bursts sit after the av1 emission
                if h == 0:
                    emit_s(3, [3, 4, 5])
                elif h + 3 < H:
                    emit_s(h + 3, [0, 1, 2, 3])
                for mo in fillers.get(h, [])[1:]:
                    emit_qk(mo)
                if h == 0:
                    emit_s(3, [6, 7])
                elif h + 3 < H:
                    emit_s(h + 3, [4, 5, 6, 7])
                if h == H - 1:
                    av1 = emit_av_mm(h, 1)
                    emit_av_norm(h, 1, av1)

        with nc.named_scope("proj"):
            for mt in range(NT):
                emit_proj(mt)

    return nc


_BUILT = None


def _get_built():
    global _BUILT
    if _BUILT is None:
        _patch_tile_drain()
        nc = build()
        _split_excess_waits(nc)
        _BUILT = nc
    return _BUILT


def kernel(x, attn_mask, qkv_w, qkv_b, proj_w, proj_b):
    x = np.asarray(x, dtype=np.float32)
    qkv_w = np.asarray(qkv_w, dtype=np.float32)
    qkv_b = np.asarray(qkv_b, dtype=np.float32)
    proj_w = np.asarray(proj_w, dtype=np.float32)
    proj_b = np.asarray(proj_b, dtype=np.float32)

    qk_w = qkv_w[: 2 * C].copy()
    qk_b = qkv_b[: 2 * C].copy()
    qk_w[:C] *= SCALE          # fold 1/sqrt(HD) into q
    qk_b[:C] *= SCALE
    v_w = qkv_w[2 * C :]
    v_b = qkv_b[2 * C :]
    qkwT = np.ascontiguousarray(qk_w.T).astype(NPF16)
    vwT = np.ascontiguousarray(v_w.T).astype(NPF16)
    pwT = np.ascontiguousarray(proj_w.T).astype(NPF16)
    pb_eff = (proj_b + proj_w @ v_b).astype(np.float32)   # v bias folded
    qkb2 = np.ascontiguousarray(qk_b.reshape(MOQ, P).T).astype(np.float32)

    nc = _get_built()
    in_maps = []
    for b in range(B):
        in_maps.append(
            {
                "xT": np.ascontiguousarray(x[b].T).astype(NPF16),
                "qkwT": qkwT,
                "vwT": vwT,
                "pwT": pwT,
                "qkb2": qkb2,
                "pb": pb_eff,
            }
        )
    res = run_bass_kernel_spmd(nc, in_maps, core_ids=list(range(B)))
    out = np.stack([res.results[b]["y"] for b in range(B)], axis=0)
    return out.astype(np.float32)
